# revision 3
# baseline (speedup 1.0000x reference)
"""Fused LoRA-MLP (SwiGLU) expert kernel for TRN2, 8-core expert-parallel.

Key ideas vs the v1 kernel:
  * LoRA branches are folded into the base weights on the host
    (W1 = W_gu + s*B_gu@A_gu, W2 = W_d + s*B_d@A_d) — mathematically
    exact, removes all LoRA matmuls and their sequencing from the device.
  * All matmul operands are bf16 (PSUM accumulation stays fp32): same
    1 cycle/row PE rate as float32r, half the HBM traffic and SBUF
    footprint. Measured end-to-end rel err ~4e-3 (threshold 2e-2).
  * mm2 accumulates all 32 contraction tiles (plus nothing else) into a
    single PSUM bank per output tile — no DVE accumulation passes.

Per core: x:(1024,2048) -> gu = x@W1.T (.,8192), h = up*silu(gate),
out = h@W2.T (.,2048). Weights replicated per core, tokens sharded.
"""

import os
from contextlib import ExitStack

import numpy as np
import ml_dtypes

import concourse.bass as bass
import concourse.bacc as bacc
import concourse.tile as tile
import concourse.mybir as mybir
from concourse.bass_utils import run_bass_kernel_spmd

F32 = mybir.dt.float32
BF16 = mybir.dt.bfloat16
AF = mybir.ActivationFunctionType
NPBF16 = ml_dtypes.bfloat16

NCORES = 8
T = 1024          # tokens per core
H = 2048          # hidden
D = 4096          # expert dim
F = 2 * D         # gate+up features
SCALING = 16 / 64

KT = H // 128     # 16 k-tiles (mm1 contraction)
FT = F // 128     # 64 mm1 output tiles
DT = D // 128     # 32 d-tiles (mm2 contraction)
JT = H // 128     # 16 mm2 output tiles
TC = 512          # moving-dim chunk (PSUM bank = 512 fp32)
NCH = T // TC     # 2 chunks

_CACHE = {}


def _build(reps=1):
    nc = bacc.Bacc("TRN2", target_bir_lowering=False, debug=False,
                   num_devices=NCORES)

    xk = nc.dram_tensor("xk", [KT, 128, T], BF16, kind="ExternalInput")
    w1 = nc.dram_tensor("w1", [FT, 128, KT * 128], BF16, kind="ExternalInput")
    w2 = nc.dram_tensor("w2", [JT, 128, DT * 128], BF16, kind="ExternalInput")
    outT = nc.dram_tensor("outT", [JT, 128, T], F32, kind="ExternalOutput")

    with tile.TileContext(nc) as tc, ExitStack() as ctx:
        xpool = ctx.enter_context(tc.tile_pool(name="xp", bufs=KT * NCH))
        w1p = ctx.enter_context(tc.tile_pool(name="w1p", bufs=6))
        w2p = ctx.enter_context(tc.tile_pool(name="w2p", bufs=3))
        htp = ctx.enter_context(tc.tile_pool(name="htp", bufs=DT))
        silp = ctx.enter_context(tc.tile_pool(name="silp", bufs=3))
        outp = ctx.enter_context(tc.tile_pool(name="outp", bufs=3))
        ps_a = ctx.enter_context(tc.tile_pool(name="psa", bufs=4, space="PSUM"))
        ps_b = ctx.enter_context(tc.tile_pool(name="psb", bufs=4, space="PSUM"))

        for rep in range(reps):
            xt = {}

            def load_x(c, k, rep=rep):
                # x rides the gpsimd DGE queue so it never serializes
                # behind weight slabs on the sync queue
                t = xpool.tile([128, TC], BF16, tag="x", name=f"x_{rep}_{c}_{k}")
                nc.gpsimd.dma_start(out=t[:], in_=xk[k, :, c * TC:(c + 1) * TC])
                xt[(c, k)] = t

            slabs = {}

            def load_w1(m, pieces=1):
                # halves ride different DGE queues (sync + scalar) so slab
                # supply is never bound by a single queue
                s = w1p.tile([128, KT * 128], BF16, tag="w1")
                half = KT * 128 // 2
                step = half // pieces
                for p in range(pieces):
                    nc.sync.dma_start(out=s[:, p * step:(p + 1) * step],
                                      in_=w1[m, :, p * step:(p + 1) * step])
                    nc.scalar.dma_start(
                        out=s[:, half + p * step:half + (p + 1) * step],
                        in_=w1[m, :, half + p * step:half + (p + 1) * step])
                slabs[m] = s

            # Issue order on the DMA queues is chosen so the PE never
            # outruns supply: slab pair 0 (first slab in pieces so the PE
            # starts on piece 0), x chunk-0 tiles in consumption order,
            # slab pair 1, x chunk-1 tiles.
            load_w1(0, pieces=2)
            load_w1(DT)
            for k in range(KT):
                load_x(0, k)
            load_w1(1)
            load_w1(1 + DT)
            for k in range(KT):
                load_x(1, k)

            if rep == 0:
                # tiny activation up front so the Silu table load happens
                # during DMA warmup, not on the first real silu
                warm = silp.tile([128, 1], BF16, tag="sil", name="warm")
                nc.scalar.activation(warm[:], xt[(0, 0)][:, 0:1], AF.Silu)

            ht = [None] * DT
            for i in range(DT):
                if i + 2 < DT:
                    load_w1(i + 2)
                    load_w1(i + 2 + DT)
                ht_i = htp.tile([128, T], BF16, tag="ht", name=f"ht_{rep}_{i}")
                ht[i] = ht_i
                sg, su = slabs.pop(i), slabs.pop(i + DT)
                for c in range(NCH):
                    pg = ps_a.tile([128, TC], F32, tag="psa")
                    pu = ps_a.tile([128, TC], F32, tag="psa")
                    for ps, s in ((pg, sg), (pu, su)):
                        for k in range(KT):
                            nc.tensor.matmul(
                                ps[:], s[:, k * 128:(k + 1) * 128],
                                xt[(c, k)][:], start=(k == 0), stop=(k == KT - 1))
                    sil = silp.tile([128, TC], BF16, tag="sil")
                    nc.scalar.activation(sil[:], pg[:], AF.Silu)
                    nc.vector.tensor_mul(
                        ht_i[:, c * TC:(c + 1) * TC], pu[:], sil[:])

            # ---- mm2: out_j = sum_d W2[j,d] @ ht[d], full PSUM accumulation
            w2t = [None] * JT

            def load_w2(j):
                t = w2p.tile([128, DT * 128], BF16, tag="w2")
                nc.sync.dma_start(out=t[:], in_=w2[j])
                w2t[j] = t

            load_w2(0)
            load_w2(1)
            load_w2(2)
            for j in range(JT):
                if j + 3 < JT:
                    load_w2(j + 3)
                ot = outp.tile([128, T], F32, tag="out")
                s = w2t[j]
                for c in range(NCH):
                    ps = ps_b.tile([128, TC], F32, tag="psb")
                    for d in range(DT):
                        nc.tensor.matmul(
                            ps[:], s[:, d * 128:(d + 1) * 128],
                            ht[d][:, c * TC:(c + 1) * TC],
                            start=(d == 0), stop=(d == DT - 1))
                    # stores ride the scalar-engine DGE queue (idle during
                    # mm2) — keeps the sync queue free for weights and
                    # shortens the end-of-kernel drain
                    if j < JT - 1 or c == 0:
                        nc.vector.tensor_copy(ot[:, c * TC:(c + 1) * TC],
                                              ps[:])
                        nc.scalar.dma_start(
                            out=outT[j, :, c * TC:(c + 1) * TC],
                            in_=ot[:, c * TC:(c + 1) * TC])
                    else:
                        # very last chunk: halves drained by two engines +
                        # two DMA queues in parallel
                        hs = TC // 2
                        lo = c * TC
                        nc.vector.tensor_copy(ot[:, lo:lo + hs],
                                              ps[:, 0:hs])
                        nc.scalar.dma_start(out=outT[j, :, lo:lo + hs],
                                            in_=ot[:, lo:lo + hs])
                        nc.scalar.activation(ot[:, lo + hs:lo + TC],
                                             ps[:, hs:], AF.Copy)
                        nc.sync.dma_start(out=outT[j, :, lo + hs:lo + TC],
                                          in_=ot[:, lo + hs:lo + TC])

    nc.compile()
    return nc


def _prep_shared(W_gu, A_gu, B_gu, W_d, A_d, B_d):
    # fold LoRA into the base weights (exact): y = x @ (W + s*B@A).T
    W1 = W_gu + SCALING * (B_gu.astype(np.float32) @ A_gu.astype(np.float32))
    W2 = W_d + SCALING * (B_d.astype(np.float32) @ A_d.astype(np.float32))
    # tile: w1_t[m, p, k*128+f] = W1[m*128+f, k*128+p]
    w1_t = np.ascontiguousarray(
        W1.astype(NPBF16).reshape(FT, 128, KT, 128).transpose(0, 3, 2, 1)
    ).reshape(FT, 128, KT * 128)
    w2_t = np.ascontiguousarray(
        W2.astype(NPBF16).reshape(JT, 128, DT, 128).transpose(0, 3, 2, 1)
    ).reshape(JT, 128, DT * 128)
    return dict(w1=w1_t, w2=w2_t)


def _prep_x(hidden_states):
    # xt[core][k, p, t] = x[core*T + t, k*128 + p]
    return np.ascontiguousarray(
        hidden_states.astype(NPBF16).reshape(NCORES, T, KT, 128)
        .transpose(0, 2, 3, 1))


def _in_maps(hidden_states, W_gu, A_gu, B_gu, W_d, A_d, B_d):
    shared = _prep_shared(
        *(np.asarray(a, dtype=np.float32)
          for a in (W_gu, A_gu, B_gu, W_d, A_d, B_d)))
    xt = _prep_x(np.asarray(hidden_states, dtype=np.float32))
    return [dict(shared, xk=xt[c]) for c in range(NCORES)]


def _unpack(res_list):
    out = np.empty((NCORES, T, H), np.float32)
    for c in range(NCORES):
        o = res_list[c]["outT"].reshape(JT, 128, T)
        out[c] = o.transpose(2, 0, 1).reshape(T, H)
    return out.reshape(NCORES * T, H)


def kernel(hidden_states, W_gu, A_gu, B_gu, W_d, A_d, B_d):
    in_maps = _in_maps(hidden_states, W_gu, A_gu, B_gu, W_d, A_d, B_d)

    if "nc" not in _CACHE:
        _CACHE["nc"] = _build()
    nc = _CACHE["nc"]

    trace = os.environ.get("KERNEL_TRACE", "0") == "1"
    res = run_bass_kernel_spmd(nc, in_maps, list(range(NCORES)), trace=trace)
    _CACHE["last_result"] = res
    return _unpack(res.results)


# revision 4
# speedup vs baseline: 1.2666x; 1.2666x over previous
"""Fused LoRA-MLP (SwiGLU) expert kernel for TRN2, 8-core expert-parallel.

Key ideas vs the v1 kernel:
  * LoRA branches are folded into the base weights on the host
    (W1 = W_gu + s*B_gu@A_gu, W2 = W_d + s*B_d@A_d) — mathematically
    exact, removes all LoRA matmuls and their sequencing from the device.
  * All matmul operands are bf16 (PSUM accumulation stays fp32): same
    1 cycle/row PE rate as float32r, half the HBM traffic and SBUF
    footprint. Measured end-to-end rel err ~4e-3 (threshold 2e-2).
  * mm2 accumulates all 32 contraction tiles (plus nothing else) into a
    single PSUM bank per output tile — no DVE accumulation passes.

Per core: x:(1024,2048) -> gu = x@W1.T (.,8192), h = up*silu(gate),
out = h@W2.T (.,2048). Weights replicated per core, tokens sharded.

Measured ~790-800us/rep on HW (PE row floor 655us + ~42ns/matmul
weight-reload exposure that survives every available lever: ldw-opt
crashes walrus, LDW dedupe and column tiling measured slower, fp8
DoubleRow fails the accuracy gate, DMA fully hidden per ablation).
"""

import os
from contextlib import ExitStack

import numpy as np
import ml_dtypes

import concourse.bass as bass
import concourse.bacc as bacc
import concourse.tile as tile
import concourse.mybir as mybir
from concourse.bass_utils import run_bass_kernel_spmd

F32 = mybir.dt.float32
BF16 = mybir.dt.bfloat16
AF = mybir.ActivationFunctionType
NPBF16 = ml_dtypes.bfloat16

NCORES = 8
T = 1024          # tokens per core
H = 2048          # hidden
D = 4096          # expert dim
F = 2 * D         # gate+up features
SCALING = 16 / 64

KT = H // 128     # 16 k-tiles (mm1 contraction)
FT = F // 128     # 64 mm1 output tiles
DT = D // 128     # 32 d-tiles (mm2 contraction)
JT = H // 128     # 16 mm2 output tiles
TC = 512          # moving-dim chunk (PSUM bank = 512 fp32)
NCH = T // TC     # 2 chunks

_CACHE = {}


def _build(reps=1):
    nc = bacc.Bacc("TRN2", target_bir_lowering=False, debug=False,
                   num_devices=NCORES)

    xk = nc.dram_tensor("xk", [KT, 128, T], BF16, kind="ExternalInput")
    w1 = nc.dram_tensor("w1", [FT, 128, KT * 128], BF16, kind="ExternalInput")
    w2 = nc.dram_tensor("w2", [JT, 128, DT * 128], BF16, kind="ExternalInput")
    outT = nc.dram_tensor("outT", [JT, 128, T], F32, kind="ExternalOutput")

    with tile.TileContext(nc) as tc, ExitStack() as ctx:
        xpool = ctx.enter_context(tc.tile_pool(name="xp", bufs=KT * NCH))
        w1p = ctx.enter_context(tc.tile_pool(name="w1p", bufs=6))
        w2p = ctx.enter_context(tc.tile_pool(name="w2p", bufs=3))
        htp = ctx.enter_context(tc.tile_pool(name="htp", bufs=DT))
        silp = ctx.enter_context(tc.tile_pool(name="silp", bufs=3))
        outp = ctx.enter_context(tc.tile_pool(name="outp", bufs=3))
        ps_a = ctx.enter_context(tc.tile_pool(name="psa", bufs=4, space="PSUM"))
        ps_b = ctx.enter_context(tc.tile_pool(name="psb", bufs=4, space="PSUM"))

        for rep in range(reps):
            xt = {}

            def load_x(c, k, rep=rep):
                # x rides the gpsimd DGE queue so it never serializes
                # behind weight slabs on the sync queue
                t = xpool.tile([128, TC], BF16, tag="x", name=f"x_{rep}_{c}_{k}")
                nc.gpsimd.dma_start(out=t[:], in_=xk[k, :, c * TC:(c + 1) * TC])
                xt[(c, k)] = t

            slabs = {}

            def load_w1(m, pieces=1):
                # halves ride different DGE queues (sync + scalar) so slab
                # supply is never bound by a single queue
                s = w1p.tile([128, KT * 128], BF16, tag="w1")
                half = KT * 128 // 2
                step = half // pieces
                for p in range(pieces):
                    nc.sync.dma_start(out=s[:, p * step:(p + 1) * step],
                                      in_=w1[m, :, p * step:(p + 1) * step])
                    nc.scalar.dma_start(
                        out=s[:, half + p * step:half + (p + 1) * step],
                        in_=w1[m, :, half + p * step:half + (p + 1) * step])
                slabs[m] = s

            # Issue order on the DMA queues is chosen so the PE never
            # outruns supply: slab pair 0 (first slab in pieces so the PE
            # starts on piece 0), x chunk-0 tiles in consumption order,
            # slab pair 1, x chunk-1 tiles.
            load_w1(0, pieces=2)
            load_w1(DT)
            for k in range(KT):
                load_x(0, k)
            load_w1(1)
            load_w1(1 + DT)
            for k in range(KT):
                load_x(1, k)

            if rep == 0:
                # tiny activation up front so the Silu table load happens
                # during DMA warmup, not on the first real silu
                warm = silp.tile([128, 1], BF16, tag="sil", name="warm")
                nc.scalar.activation(warm[:], xt[(0, 0)][:, 0:1], AF.Silu)

            ht = [None] * DT
            for i in range(DT):
                if i + 2 < DT:
                    load_w1(i + 2)
                    load_w1(i + 2 + DT)
                ht_i = htp.tile([128, T], BF16, tag="ht", name=f"ht_{rep}_{i}")
                ht[i] = ht_i
                sg, su = slabs.pop(i), slabs.pop(i + DT)
                for c in range(NCH):
                    pg = ps_a.tile([128, TC], F32, tag="psa")
                    pu = ps_a.tile([128, TC], F32, tag="psa")
                    for ps, s in ((pg, sg), (pu, su)):
                        for k in range(KT):
                            nc.tensor.matmul(
                                ps[:], s[:, k * 128:(k + 1) * 128],
                                xt[(c, k)][:], start=(k == 0), stop=(k == KT - 1))
                    sil = silp.tile([128, TC], BF16, tag="sil")
                    nc.scalar.activation(sil[:], pg[:], AF.Silu)
                    nc.vector.tensor_mul(
                        ht_i[:, c * TC:(c + 1) * TC], pu[:], sil[:])

            # ---- mm2: out_j = sum_d W2[j,d] @ ht[d], full PSUM accumulation
            w2t = [None] * JT

            def load_w2(j):
                t = w2p.tile([128, DT * 128], BF16, tag="w2")
                nc.sync.dma_start(out=t[:], in_=w2[j])
                w2t[j] = t

            load_w2(0)
            load_w2(1)
            load_w2(2)
            for j in range(JT):
                if j + 3 < JT:
                    load_w2(j + 3)
                ot = outp.tile([128, T], F32, tag="out")
                s = w2t[j]
                for c in range(NCH):
                    ps = ps_b.tile([128, TC], F32, tag="psb")
                    for d in range(DT):
                        nc.tensor.matmul(
                            ps[:], s[:, d * 128:(d + 1) * 128],
                            ht[d][:, c * TC:(c + 1) * TC],
                            start=(d == 0), stop=(d == DT - 1))
                    # stores ride the scalar-engine DGE queue (idle during
                    # mm2) — keeps the sync queue free for weights and
                    # shortens the end-of-kernel drain
                    if j < JT - 1 or c == 0:
                        nc.vector.tensor_copy(ot[:, c * TC:(c + 1) * TC],
                                              ps[:])
                        nc.scalar.dma_start(
                            out=outT[j, :, c * TC:(c + 1) * TC],
                            in_=ot[:, c * TC:(c + 1) * TC])
                    else:
                        # very last chunk: halves drained by two engines +
                        # two DMA queues in parallel
                        hs = TC // 2
                        lo = c * TC
                        nc.vector.tensor_copy(ot[:, lo:lo + hs],
                                              ps[:, 0:hs])
                        nc.scalar.dma_start(out=outT[j, :, lo:lo + hs],
                                            in_=ot[:, lo:lo + hs])
                        nc.scalar.activation(ot[:, lo + hs:lo + TC],
                                             ps[:, hs:], AF.Copy)
                        nc.sync.dma_start(out=outT[j, :, lo + hs:lo + TC],
                                          in_=ot[:, lo + hs:lo + TC])

    nc.compile()
    return nc


def _prep_shared(W_gu, A_gu, B_gu, W_d, A_d, B_d):
    # fold LoRA into the base weights (exact): y = x @ (W + s*B@A).T
    W1 = W_gu + SCALING * (B_gu.astype(np.float32) @ A_gu.astype(np.float32))
    W2 = W_d + SCALING * (B_d.astype(np.float32) @ A_d.astype(np.float32))
    # tile: w1_t[m, p, k*128+f] = W1[m*128+f, k*128+p]
    w1_t = np.ascontiguousarray(
        W1.astype(NPBF16).reshape(FT, 128, KT, 128).transpose(0, 3, 2, 1)
    ).reshape(FT, 128, KT * 128)
    w2_t = np.ascontiguousarray(
        W2.astype(NPBF16).reshape(JT, 128, DT, 128).transpose(0, 3, 2, 1)
    ).reshape(JT, 128, DT * 128)
    return dict(w1=w1_t, w2=w2_t)


def _prep_x(hidden_states):
    # xt[core][k, p, t] = x[core*T + t, k*128 + p]
    return np.ascontiguousarray(
        hidden_states.astype(NPBF16).reshape(NCORES, T, KT, 128)
        .transpose(0, 2, 3, 1))


def _in_maps(hidden_states, W_gu, A_gu, B_gu, W_d, A_d, B_d):
    shared = _prep_shared(
        *(np.asarray(a, dtype=np.float32)
          for a in (W_gu, A_gu, B_gu, W_d, A_d, B_d)))
    xt = _prep_x(np.asarray(hidden_states, dtype=np.float32))
    return [dict(shared, xk=xt[c]) for c in range(NCORES)]


def _unpack(res_list):
    out = np.empty((NCORES, T, H), np.float32)
    for c in range(NCORES):
        o = res_list[c]["outT"].reshape(JT, 128, T)
        out[c] = o.transpose(2, 0, 1).reshape(T, H)
    return out.reshape(NCORES * T, H)


def kernel(hidden_states, W_gu, A_gu, B_gu, W_d, A_d, B_d):
    in_maps = _in_maps(hidden_states, W_gu, A_gu, B_gu, W_d, A_d, B_d)

    if "nc" not in _CACHE:
        _CACHE["nc"] = _build()
    nc = _CACHE["nc"]

    trace = os.environ.get("KERNEL_TRACE", "0") == "1"
    res = run_bass_kernel_spmd(nc, in_maps, list(range(NCORES)), trace=trace)
    _CACHE["last_result"] = res
    return _unpack(res.results)
